# revision 11
# baseline (speedup 1.0000x reference)
"""GATv2 self-attention kernel for 8 Trainium2 NeuronCores.

Sharding: one attention head per core (8 heads / 8 cores). Each core computes
its head's attn-weighted projection as a partial sum over heads, the cores
ReduceScatter the partials over the feature axis (bf16), and each core
finishes its 256-column feature slice (bias-mean + residual) and returns it;
the host concatenates the 8 slices.

Math per head h (reference):
  X = inputs.reshape(B*S, F); x0 = rows of X with s == 0
  Wh = leaky_relu(X @ W2h + broadcast_s(x0 @ W1h))      [B*S, F]
  e  = Wh @ att_w[h]; attn = softmax_s(e)
  out = sum_h (attn * Wh)/H + mean_h(bias) + X

The heavy contractions run in fp8 (e4m3) with MatmulPerfMode.DoubleRow
(K=256 per PE instruction, ~2-3x bf16 MAC throughput). W is pre-scaled by
WSCALE=64 on the host so its values sit in e4m3's normal range; the Prelu
activation unscales by 1/WSCALE when writing Wh (bf16). The broadcast
x0@W1 term is accumulated into the same PSUM group through a DoubleRow
selector matmul (sel = 2.0 per slot at (b%128, bs)) against the on-chip
X0 = x0@W1, which is stored fp8 at 16x scale and duplicated along the DR
pair axis (16*2*2 = WSCALE). X arrives pre-transposed from the host (fp8),
so the PE does no input transposes. The e-matvec also runs as DR fp8 off
an fp8 copy of Wh, with attw padded to 16 columns (dual-fp8 ldweights
requires >=16 active columns); psum row 0 carries e. Work is split into
4 bs-chunks of 512 so each chunk's bf16 ReduceScatter overlaps the next
chunk's compute.
"""

import sys
import numpy as np

sys.path.insert(0, "/opt/trn_rl_repo")

B, S, F, H = 256, 8, 2048, 8
BS = B * S            # 2048
NB = 512              # bs-chunk size
NCHUNK = BS // NB     # 4
FSLICE = F // H       # 256 output feature columns per core
NFB = F // 128        # 16 feature blocks
NKP = NFB // 2        # 8 DoubleRow K-pairs
ALPHA = 0.3
WSCALE = 64.0         # host pre-scale on W and att_w for fp8 range
X0SCALE = 16.0        # on-chip storage scale of X0 (selector rhs supplies x4)

_cache = {}


def _build(reps=1):
    import concourse.bacc as bacc
    import concourse.mybir as mybir
    import concourse.tile as tile
    import concourse.bass as bass
    from concourse.masks import make_identity

    f32 = mybir.dt.float32
    bf16 = mybir.dt.bfloat16
    f8 = mybir.dt.float8e4
    AF = mybir.ActivationFunctionType
    OP = mybir.AluOpType

    nc = bacc.Bacc(num_devices=H)

    w2_in = nc.declare_dram_parameter("w2t", [128, NKP, 2, NFB, 128], f8, isOutput=False)
    w1_in = nc.declare_dram_parameter("w1t", [4, 128, NKP, 2, 512], f8, isOutput=False)
    xt_in = nc.declare_dram_parameter("xt", [128, NKP, 2, BS], f8, isOutput=False)
    x0t_in = nc.declare_dram_parameter("x0t", [128, NKP, 2, B], f8, isOutput=False)
    attw8_in = nc.declare_dram_parameter("attw8", [128, NFB, 16], f8, isOutput=False)
    attwb_in = nc.declare_dram_parameter("attwb", [128, NFB], bf16, isOutput=False)
    sel_in = nc.declare_dram_parameter("sel4", [2, 2, 128, NB], f8, isOutput=False)
    xres_in = nc.declare_dram_parameter("xres", [BS, FSLICE], f32, isOutput=False)
    bm_in = nc.declare_dram_parameter("bm", [FSLICE], f32, isOutput=False)
    out_ext = nc.declare_dram_parameter("out", [BS, FSLICE], f32, isOutput=True)

    from contextlib import ExitStack
    with tile.TileContext(nc) as tc:
        with ExitStack() as ctx:
            consts = ctx.enter_context(tc.tile_pool(name="consts", bufs=1))
            w2p = ctx.enter_context(tc.tile_pool(name="w2p", bufs=1))
            xtp = ctx.enter_context(tc.tile_pool(name="xtp", bufs=1))
            w1p = ctx.enter_context(tc.tile_pool(name="w1p", bufs=2))
            x0p = ctx.enter_context(tc.tile_pool(name="x0p", bufs=1))
            whp = ctx.enter_context(tc.tile_pool(name="whp", bufs=2))
            esmp = ctx.enter_context(tc.tile_pool(name="esm", bufs=2))
            abp = ctx.enter_context(tc.tile_pool(name="abp", bufs=2))
            rsbp = ctx.enter_context(tc.tile_pool(name="rsbp", bufs=2))
            xrsp = ctx.enter_context(tc.tile_pool(name="xrs", bufs=2))
            outstp = ctx.enter_context(tc.tile_pool(name="outst", bufs=2))
            ypool = ctx.enter_context(tc.tile_pool(name="ypool", bufs=4, space="PSUM"))
            epool = ctx.enter_context(tc.tile_pool(name="epool", bufs=2, space="PSUM"))
            tpool = ctx.enter_context(tc.tile_pool(name="tpool", bufs=2, space="PSUM"))
            dpool = ctx.enter_context(tc.tile_pool(name="dram", bufs=4, space="DRAM"))

            # ---------------- constants ----------------
            ident_bf = consts.tile([128, 128], bf16)
            make_identity(nc, ident_bf)

            attw8 = consts.tile([128, NFB, 16], f8)
            nc.sync.dma_start(out=attw8, in_=attw8_in[:, :, :])
            attwb = consts.tile([128, NFB], bf16)
            nc.sync.dma_start(out=attwb, in_=attwb_in[:, :])

            selq = consts.tile([128, 2, 2, NB], f8)
            nc.sync.dma_start(out=selq, in_=sel_in.rearrange("t u p n -> p t u n"))

            al_sb = consts.tile([128, 1], f32)
            nc.vector.memset(al_sb, ALPHA)

            bm_sb = consts.tile([128, FSLICE // 128], f32)
            nc.sync.dma_start(out=bm_sb, in_=bm_in.rearrange("(o p) -> p o", p=128))

            for _rep in range(reps):
                _run_body(nc, tc, mybir, bass, f32, bf16, f8, AF, OP,
                          ident_bf, attw8, attwb, selq, bm_sb, al_sb,
                          w2_in, w1_in, xt_in, x0t_in, xres_in, out_ext,
                          w2p, xtp, w1p, x0p, whp, esmp, abp,
                          rsbp, xrsp, outstp, ypool, epool, tpool, dpool, _rep)

    nc.compile()
    return nc


def _run_body(nc, tc, mybir, bass, f32, bf16, f8, AF, OP,
              ident_bf, attw8, attwb, selq, bm_sb, al_sb,
              w2_in, w1_in, xt_in, x0t_in, xres_in, out_ext,
              w2p, xtp, w1p, x0p, whp, esmp, abp,
              rsbp, xrsp, outstp, ypool, epool, tpool, dpool, rep):
    DR = mybir.MatmulPerfMode.DoubleRow

    # ---------------- resident loads ----------------
    w2sb = w2p.tile([128, NKP, 2, NFB, 128], f8, tag="w2")
    nc.sync.dma_start(out=w2sb, in_=w2_in[:, :, :, :, :])
    xtsb = xtp.tile([128, NKP, 2, BS], f8, tag="xt")
    nc.sync.dma_start(out=xtsb, in_=xt_in[:, :, :, :])
    x0tsb = x0p.tile([128, NKP, 2, B], f8, tag="x0t", name=f"x0t{rep}")
    nc.sync.dma_start(out=x0tsb, in_=x0t_in[:, :, :, :])

    # ---------------- prologue: X0 = x0 @ W1, stored fp8 at X0SCALE ----------
    # X0q: [128 b_in, 2 b_out, 16 fo_out, 2 dup, 128 fo_in] -- X0 duplicated
    # along the DoubleRow pair axis so the selector matmul runs in DR mode
    # (sel supplies 2.0 per slot; 16 * 2 * 2 = WSCALE).
    x0q = x0p.tile([128, 2, NFB, 2, 128], f8, tag="x0q", name=f"x0q{rep}")
    for f4 in range(4):
        w1blk = w1p.tile([128, NKP, 2, 512], f8, tag="w1blk")
        nc.sync.dma_start(out=w1blk, in_=w1_in[f4])
        for bb in range(2):
            ps = ypool.tile([128, 512], f32, tag="yp")
            for k in range(NKP):
                nc.tensor.matmul(
                    ps, x0tsb[:, k, :, bb * 128:(bb + 1) * 128],
                    w1blk[:, k, :, :],
                    start=(k == 0), stop=(k == NKP - 1), perf_mode=DR)
            # psum holds WSCALE * X0; store fp8 at X0SCALE, twice
            for u in range(2):
                nc.scalar.activation(
                    x0q[:, bb, 4 * f4:4 * f4 + 4, u, :], ps, AF.Copy,
                    scale=X0SCALE / WSCALE)

    # ---------------- main loop over bs-chunks ----------------
    rs_list = []
    for c in range(NCHUNK):
        wh = whp.tile([128, NFB, NB], bf16, tag="wh")
        wh8 = whp.tile([128, NFB, NB], f8, tag="wh8")
        e_ps = epool.tile([16, NB], f32, tag="ep", name=f"eps{rep}_{c}")

        pending_e = []  # lag e-matvecs so PE never waits on ACT
        for fo in range(NFB):
            ps = ypool.tile([128, 512], f32, tag="yp")
            for k in range(NKP):
                nc.tensor.matmul(
                    ps, w2sb[:, k, :, fo, :],
                    xtsb[:, k, :, c * NB:(c + 1) * NB],
                    start=(k == 0), stop=False, perf_mode=DR)
            # + broadcast_s(X0): DR selector matmul closes the group.
            # lhsT = X0q dup-pair (fp8, 16x), rhs = sel (2.0 at (b%128, bs))
            nc.tensor.matmul(
                ps, x0q[:, c // 2, fo, :, :], selq[:, c % 2, :, :],
                start=False, stop=True, perf_mode=DR)
            # leaky_relu via Prelu, unscale by 1/WSCALE, write bf16 Wh
            nc.scalar.activation(wh[:, fo, :], ps, AF.Prelu,
                                 scale=1.0 / WSCALE, alpha=al_sb[:, :])
            # fp8 copy of Wh for the DR e-matvec
            nc.scalar.activation(wh8[:, fo, :], wh[:, fo, :], AF.Copy)
            if fo % 2 == 1:
                pending_e.append(fo // 2)
            # emit lagged DR e-matvecs (one pair back)
            while len(pending_e) > 1:
                ej = pending_e.pop(0)
                nc.tensor.matmul(
                    e_ps, attw8[:, 2 * ej:2 * ej + 2, :],
                    wh8[:, 2 * ej:2 * ej + 2, :],
                    start=(ej == 0), stop=(ej == NKP - 1), perf_mode=DR)
        for ej in pending_e:
            nc.tensor.matmul(
                e_ps, attw8[:, 2 * ej:2 * ej + 2, :],
                wh8[:, 2 * ej:2 * ej + 2, :],
                start=(ej == 0), stop=(ej == NKP - 1), perf_mode=DR)

        # -- softmax over s (groups of 8 along bs), scaled by 1/H --
        # e_ps holds WSCALE * e (attw8 is pre-scaled); unscale on copy
        NG = NB // S
        e_sb = esmp.tile([1, NB], f32, tag="esb", name=f"esb{rep}_{c}")
        nc.scalar.activation(e_sb, e_ps[0:1, :], AF.Copy, scale=1.0 / WSCALE)
        work = esmp.tile([1, NB], f32, tag="work", name=f"work{rep}_{c}")
        e3 = e_sb.rearrange("p (b s) -> p b s", s=S)
        w3 = work.rearrange("p (b s) -> p b s", s=S)
        mx = esmp.tile([1, NG], f32, tag="mx", name=f"mx{rep}_{c}")
        nc.vector.reduce_max(out=mx, in_=e3, axis=mybir.AxisListType.X)
        nc.vector.tensor_tensor(
            out=w3, in0=e3, in1=mx[:, :, None].to_broadcast((1, NG, S)),
            op=OP.subtract)
        nc.scalar.activation(e_sb, work, AF.Exp)
        sm = esmp.tile([1, NG], f32, tag="sm", name=f"sm{rep}_{c}")
        nc.vector.reduce_sum(out=sm, in_=e3, axis=mybir.AxisListType.X)
        rc = esmp.tile([1, NG], f32, tag="rc", name=f"rc{rep}_{c}")
        nc.vector.reciprocal(rc, sm)
        nc.vector.tensor_scalar_mul(rc, rc, 1.0 / H)
        attn_sb = work
        nc.vector.tensor_tensor(
            out=w3, in0=e3, in1=rc[:, :, None].to_broadcast((1, NG, S)),
            op=OP.mult)
        # broadcast attn across partitions via DRAM round-trip
        attn_dram = dpool.tile([1, NB], f32, tag="attn_dram")
        nc.gpsimd.dma_start(out=attn_dram[:, :], in_=attn_sb)
        attn_bc = bass.AP(
            tensor=attn_dram.tensor,
            offset=attn_dram.offset,
            ap=[[0, 128]] + [list(p) for p in attn_dram[:, :].ap[1:]],
        )
        ab_sb = abp.tile([128, NB], f32, tag="ab", name=f"ab{rep}_{c}")
        nc.gpsimd.dma_start(out=ab_sb, in_=attn_bc)

        # -- partial = attn/H * Wh, in place (bf16), then bulk DMA --
        for fo in range(NFB):
            nc.vector.tensor_tensor(
                out=wh[:, fo, :], in0=wh[:, fo, :], in1=ab_sb, op=OP.mult)
        partial_c = dpool.tile([F, NB], bf16, tag="partial")
        nc.gpsimd.dma_start(
            out=partial_c.rearrange("(o p) n -> p o n", p=128), in_=wh)

        # -- reduce over heads: ReduceScatter along f-axis (bf16) --
        rs_c = dpool.tile([FSLICE, NB], bf16, tag="rs", name=f"rs{rep}_{c}")
        nc.gpsimd.collective_compute(
            "ReduceScatter", OP.add,
            replica_groups=[list(range(H))],
            ins=[partial_c[:, :]], outs=[rs_c[:, :]])
        rs_list.append(rs_c)

    # finish phase after all chunks (overlaps trailing collectives)
    for c in range(NCHUNK):
        rs_c = rs_list[c]
        rsb = rsbp.tile([128, FSLICE // 128, NB], bf16, tag="rsb",
                        name=f"rsb{rep}_{c}")
        nc.sync.dma_start(
            out=rsb, in_=rs_c.rearrange("(o p) n -> p o n", p=128))
        for j in range(FSLICE // 128):
            nc.scalar.activation(
                rsb[:, j, :], rsb[:, j, :], AF.Identity,
                bias=bm_sb[:, j:j + 1])
        xrs = xrsp.tile([128, NB // 128, FSLICE], f32, tag="xrs")
        nc.sync.dma_start(
            out=xrs,
            in_=xres_in[c * NB:(c + 1) * NB, :].rearrange(
                "(o p) f -> p o f", p=128))
        obl = outstp.tile([128, NB // 128, FSLICE], f32, tag="obl")
        for bsub in range(NB // 128):
            for j in range(FSLICE // 128):
                pt = tpool.tile([128, 128], bf16, tag="tp")
                nc.tensor.transpose(
                    pt, rsb[:, j, bsub * 128:(bsub + 1) * 128], ident_bf)
                nc.vector.tensor_tensor(
                    out=obl[:, bsub, j * 128:(j + 1) * 128], in0=pt,
                    in1=xrs[:, bsub, j * 128:(j + 1) * 128], op=OP.add)
        nc.gpsimd.dma_start(
            out=out_ext[c * NB:(c + 1) * NB, :].rearrange(
                "(o p) f -> p o f", p=128),
            in_=obl)


def _get_nc():
    if "nc" not in _cache:
        _cache["nc"] = _build()
    return _cache["nc"]


def _make_in_maps(inputs_dict):
    import ml_dtypes
    f8 = ml_dtypes.float8_e4m3
    bf = ml_dtypes.bfloat16

    x = np.ascontiguousarray(
        np.asarray(inputs_dict["inputs"], dtype=np.float32).reshape(BS, F))
    W = np.asarray(inputs_dict["W"], dtype=np.float32)
    att_w = np.asarray(inputs_dict["att_w"], dtype=np.float32)
    bias = np.asarray(inputs_dict["bias"], dtype=np.float32)

    bm_full = bias.mean(axis=0)  # [F]

    # selector: 2.0 at (parity*64 + bs//S) % 128, duplicated along the
    # DoubleRow pair axis (X0q is likewise duplicated: 16 * 2 * 2 = 64)
    sel = np.zeros((2, 2, 128, NB), np.float32)
    for par in range(2):
        for j in range(NB):
            sel[par, :, par * 64 + j // S, j] = 2.0
    sel = sel.astype(f8)

    # X transposed: [F, BS] -> [128 fi_in, NKP k, 2 two, BS]
    # fi = (2k + two)*128 + fi_in
    xT8 = x.T.astype(f8)
    xt = np.ascontiguousarray(
        xT8.reshape(NKP, 2, 128, BS).transpose(2, 0, 1, 3))
    x0T8 = x[0::S, :].T.astype(f8)   # [F, B]
    x0t = np.ascontiguousarray(
        x0T8.reshape(NKP, 2, 128, B).transpose(2, 0, 1, 3))

    W8 = (W * np.float32(WSCALE)).astype(f8)  # [H, 2F, F]

    in_maps = []
    for i in range(H):
        W1 = W8[i, :F]   # [fi, fo]
        W2 = W8[i, F:]
        # w2t: [128 fi_in, NKP, 2, NFB fo_out, 128 fo_in]
        w2t = np.ascontiguousarray(
            W2.reshape(NKP, 2, 128, NFB, 128).transpose(2, 0, 1, 3, 4))
        # w1t: [4 fo4, 128 fi_in, NKP, 2, 512 fo_in]
        w1t = np.ascontiguousarray(
            W1.reshape(NKP, 2, 128, 4, 512).transpose(3, 2, 0, 1, 4))
        aw = att_w[i]  # [F]; fo = fo_out*128 + fo_in
        attw8 = np.zeros((128, NFB, 16), f8)
        attw8[:, :, 0] = (aw * np.float32(WSCALE)).astype(f8).reshape(NFB, 128).T
        attwb = np.ascontiguousarray(aw.astype(bf).reshape(NFB, 128).T)
        in_maps.append({
            "w2t": w2t,
            "w1t": w1t,
            "xt": xt,
            "x0t": x0t,
            "attw8": attw8,
            "attwb": attwb,
            "sel4": sel,
            "xres": np.ascontiguousarray(x[:, FSLICE * i:FSLICE * (i + 1)]),
            "bm": np.ascontiguousarray(bm_full[FSLICE * i:FSLICE * (i + 1)]),
        })
    return in_maps


def kernel(inputs, W, att_w, bias):
    from concourse.bass_utils import run_bass_kernel_spmd

    nc = _get_nc()
    in_maps = _make_in_maps(
        {"inputs": inputs, "W": W, "att_w": att_w, "bias": bias})
    res = run_bass_kernel_spmd(nc, in_maps, list(range(H)))
    _cache["last_result"] = res

    out = np.concatenate([res.results[i]["out"] for i in range(H)], axis=1)
    return out.reshape(B, S, F)


# revision 13
# speedup vs baseline: 2.0868x; 2.0868x over previous
"""GATv2 self-attention kernel for 8 Trainium2 NeuronCores.

Sharding: one attention head per core (8 heads / 8 cores). Each core computes
its head's attn-weighted projection as a partial sum over heads, the cores
ReduceScatter the partials over the feature axis (bf16), and each core
finishes its 256-column feature slice (bias-mean + residual) and returns it;
the host concatenates the 8 slices.

Math per head h (reference):
  X = inputs.reshape(B*S, F); x0 = rows of X with s == 0
  Wh = leaky_relu(X @ W2h + broadcast_s(x0 @ W1h))      [B*S, F]
  e  = Wh @ att_w[h]; attn = softmax_s(e)
  out = sum_h (attn * Wh)/H + mean_h(bias) + X

The heavy contractions run in fp8 (e4m3) with MatmulPerfMode.DoubleRow
(K=256 per PE instruction, ~2-3x bf16 MAC throughput). W is pre-scaled by
WSCALE=64 on the host so its values sit in e4m3's normal range; the Prelu
activation unscales by 1/WSCALE when writing Wh (bf16). The broadcast
x0@W1 term is accumulated into the same PSUM group through a DoubleRow
selector matmul (sel = 2.0 per slot at (b%128, bs)) against the on-chip
X0 = x0@W1, which is stored fp8 at 16x scale and duplicated along the DR
pair axis (16*2*2 = WSCALE). X arrives pre-transposed from the host (fp8),
so the PE does no input transposes. The e-matvec also runs as DR fp8 off
an fp8 copy of Wh, with attw padded to 16 columns (dual-fp8 ldweights
requires >=16 active columns); psum row 0 carries e. Work is split into
4 bs-chunks of 512 so each chunk's bf16 ReduceScatter overlaps the next
chunk's compute.
"""

import sys
import numpy as np

sys.path.insert(0, "/opt/trn_rl_repo")

B, S, F, H = 256, 8, 2048, 8
BS = B * S            # 2048
NB = 512              # bs-chunk size
NCHUNK = BS // NB     # 4
FSLICE = F // H       # 256 output feature columns per core
NFB = F // 128        # 16 feature blocks
NKP = NFB // 2        # 8 DoubleRow K-pairs
ALPHA = 0.3
WSCALE = 64.0         # host pre-scale on W and att_w for fp8 range
X0SCALE = 16.0        # on-chip storage scale of X0 (selector rhs supplies x4)
PSCALE = 16.0         # storage scale of fp8 partials (avoids e4m3 subnormals)

_cache = {}


def _build(reps=1):
    import concourse.bacc as bacc
    import concourse.mybir as mybir
    import concourse.tile as tile
    import concourse.bass as bass
    from concourse.masks import make_identity

    f32 = mybir.dt.float32
    bf16 = mybir.dt.bfloat16
    f8 = mybir.dt.float8e4
    AF = mybir.ActivationFunctionType
    OP = mybir.AluOpType

    nc = bacc.Bacc(num_devices=H)

    w2_in = nc.declare_dram_parameter("w2t", [128, NKP, 2, NFB, 128], f8, isOutput=False)
    w1_in = nc.declare_dram_parameter("w1t", [4, 128, NKP, 2, 512], f8, isOutput=False)
    xt_in = nc.declare_dram_parameter("xt", [128, NKP, 2, BS], f8, isOutput=False)
    x0t_in = nc.declare_dram_parameter("x0t", [128, NKP, 2, B], f8, isOutput=False)
    attw8_in = nc.declare_dram_parameter("attw8", [128, NFB, 16], f8, isOutput=False)
    attwb_in = nc.declare_dram_parameter("attwb", [128, NFB], bf16, isOutput=False)
    sel_in = nc.declare_dram_parameter("sel4", [2, 2, 128, NB], f8, isOutput=False)
    xres_in = nc.declare_dram_parameter("xres", [BS, FSLICE], f32, isOutput=False)
    bm_in = nc.declare_dram_parameter("bm", [FSLICE], f32, isOutput=False)
    out_ext = nc.declare_dram_parameter("out", [BS, FSLICE], f32, isOutput=True)

    from contextlib import ExitStack
    with tile.TileContext(nc) as tc:
        with ExitStack() as ctx:
            consts = ctx.enter_context(tc.tile_pool(name="consts", bufs=1))
            w2p = ctx.enter_context(tc.tile_pool(name="w2p", bufs=1))
            xtp = ctx.enter_context(tc.tile_pool(name="xtp", bufs=1))
            w1p = ctx.enter_context(tc.tile_pool(name="w1p", bufs=2))
            x0p = ctx.enter_context(tc.tile_pool(name="x0p", bufs=1))
            whp = ctx.enter_context(tc.tile_pool(name="whp", bufs=2))
            esmp = ctx.enter_context(tc.tile_pool(name="esm", bufs=2))
            abp = ctx.enter_context(tc.tile_pool(name="abp", bufs=2))
            rsbp = ctx.enter_context(tc.tile_pool(name="rsbp", bufs=2))
            xrsp = ctx.enter_context(tc.tile_pool(name="xrs", bufs=2))
            outstp = ctx.enter_context(tc.tile_pool(name="outst", bufs=2))
            ypool = ctx.enter_context(tc.tile_pool(name="ypool", bufs=4, space="PSUM"))
            epool = ctx.enter_context(tc.tile_pool(name="epool", bufs=2, space="PSUM"))
            tpool = ctx.enter_context(tc.tile_pool(name="tpool", bufs=2, space="PSUM"))
            dpool = ctx.enter_context(tc.tile_pool(name="dram", bufs=4, space="DRAM"))

            # ---------------- constants ----------------
            ident_bf = consts.tile([128, 128], bf16)
            make_identity(nc, ident_bf)

            attw8 = consts.tile([128, NFB, 16], f8)
            nc.sync.dma_start(out=attw8, in_=attw8_in[:, :, :])
            attwb = consts.tile([128, NFB], bf16)
            nc.sync.dma_start(out=attwb, in_=attwb_in[:, :])

            selq = consts.tile([128, 2, 2, NB], f8)
            nc.sync.dma_start(out=selq, in_=sel_in.rearrange("t u p n -> p t u n"))

            al_sb = consts.tile([128, 1], f32)
            nc.vector.memset(al_sb, ALPHA)

            bm_sb = consts.tile([128, FSLICE // 128], f32)
            nc.sync.dma_start(out=bm_sb, in_=bm_in.rearrange("(o p) -> p o", p=128))

            for _rep in range(reps):
                _run_body(nc, tc, mybir, bass, f32, bf16, f8, AF, OP,
                          ident_bf, attw8, attwb, selq, bm_sb, al_sb,
                          w2_in, w1_in, xt_in, x0t_in, xres_in, out_ext,
                          w2p, xtp, w1p, x0p, whp, esmp, abp,
                          rsbp, xrsp, outstp, ypool, epool, tpool, dpool, _rep)

    nc.compile()
    return nc


def _run_body(nc, tc, mybir, bass, f32, bf16, f8, AF, OP,
              ident_bf, attw8, attwb, selq, bm_sb, al_sb,
              w2_in, w1_in, xt_in, x0t_in, xres_in, out_ext,
              w2p, xtp, w1p, x0p, whp, esmp, abp,
              rsbp, xrsp, outstp, ypool, epool, tpool, dpool, rep):
    DR = mybir.MatmulPerfMode.DoubleRow

    # ---------------- resident loads ----------------
    w2sb = w2p.tile([128, NKP, 2, NFB, 128], f8, tag="w2")
    nc.sync.dma_start(out=w2sb, in_=w2_in[:, :, :, :, :])
    xtsb = xtp.tile([128, NKP, 2, BS], f8, tag="xt")
    nc.sync.dma_start(out=xtsb, in_=xt_in[:, :, :, :])
    x0tsb = x0p.tile([128, NKP, 2, B], f8, tag="x0t", name=f"x0t{rep}")
    nc.sync.dma_start(out=x0tsb, in_=x0t_in[:, :, :, :])

    # ---------------- prologue: X0 = x0 @ W1, stored fp8 at X0SCALE ----------
    # X0q: [128 b_in, 2 b_out, 16 fo_out, 2 dup, 128 fo_in] -- X0 duplicated
    # along the DoubleRow pair axis so the selector matmul runs in DR mode
    # (sel supplies 2.0 per slot; 16 * 2 * 2 = WSCALE).
    x0q = x0p.tile([128, 2, NFB, 2, 128], f8, tag="x0q", name=f"x0q{rep}")
    for f4 in range(4):
        w1blk = w1p.tile([128, NKP, 2, 512], f8, tag="w1blk")
        nc.sync.dma_start(out=w1blk, in_=w1_in[f4])
        for bb in range(2):
            ps = ypool.tile([128, 512], f32, tag="yp")
            for k in range(NKP):
                nc.tensor.matmul(
                    ps, x0tsb[:, k, :, bb * 128:(bb + 1) * 128],
                    w1blk[:, k, :, :],
                    start=(k == 0), stop=(k == NKP - 1), perf_mode=DR)
            # psum holds WSCALE * X0; store fp8 at X0SCALE, twice
            for u in range(2):
                nc.scalar.activation(
                    x0q[:, bb, 4 * f4:4 * f4 + 4, u, :], ps, AF.Copy,
                    scale=X0SCALE / WSCALE)

    # ---------------- main loop over bs-chunks ----------------
    rs_list = []
    for c in range(NCHUNK):
        wh8 = whp.tile([128, NFB, NB], f8, tag="wh8")
        e_ps = epool.tile([16, NB], f32, tag="ep", name=f"eps{rep}_{c}")

        pending_e = []  # lag e-matvecs so PE never waits on ACT
        for fo in range(NFB):
            ps = ypool.tile([128, 512], f32, tag="yp")
            for k in range(NKP):
                nc.tensor.matmul(
                    ps, w2sb[:, k, :, fo, :],
                    xtsb[:, k, :, c * NB:(c + 1) * NB],
                    start=(k == 0), stop=False, perf_mode=DR)
            # + broadcast_s(X0): DR selector matmul closes the group.
            # lhsT = X0q dup-pair (fp8, 16x), rhs = sel (2.0 at (b%128, bs))
            nc.tensor.matmul(
                ps, x0q[:, c // 2, fo, :, :], selq[:, c % 2, :, :],
                start=False, stop=True, perf_mode=DR)
            # leaky_relu via Prelu, unscale by 1/WSCALE, write fp8 Wh
            nc.scalar.activation(wh8[:, fo, :], ps, AF.Prelu,
                                 scale=1.0 / WSCALE, alpha=al_sb[:, :])
            if fo % 2 == 1:
                pending_e.append(fo // 2)
            # emit lagged DR e-matvecs (one pair back)
            while len(pending_e) > 1:
                ej = pending_e.pop(0)
                nc.tensor.matmul(
                    e_ps, attw8[:, 2 * ej:2 * ej + 2, :],
                    wh8[:, 2 * ej:2 * ej + 2, :],
                    start=(ej == 0), stop=(ej == NKP - 1), perf_mode=DR)
        for ej in pending_e:
            nc.tensor.matmul(
                e_ps, attw8[:, 2 * ej:2 * ej + 2, :],
                wh8[:, 2 * ej:2 * ej + 2, :],
                start=(ej == 0), stop=(ej == NKP - 1), perf_mode=DR)

        # -- softmax over s (groups of 8 along bs), scaled by 1/H --
        # e_ps holds WSCALE * e (attw8 is pre-scaled); unscale on copy
        NG = NB // S
        e_sb = esmp.tile([1, NB], f32, tag="esb", name=f"esb{rep}_{c}")
        nc.scalar.activation(e_sb, e_ps[0:1, :], AF.Copy, scale=1.0 / WSCALE)
        work = esmp.tile([1, NB], f32, tag="work", name=f"work{rep}_{c}")
        e3 = e_sb.rearrange("p (b s) -> p b s", s=S)
        w3 = work.rearrange("p (b s) -> p b s", s=S)
        mx = esmp.tile([1, NG], f32, tag="mx", name=f"mx{rep}_{c}")
        nc.vector.reduce_max(out=mx, in_=e3, axis=mybir.AxisListType.X)
        nc.vector.tensor_tensor(
            out=w3, in0=e3, in1=mx[:, :, None].to_broadcast((1, NG, S)),
            op=OP.subtract)
        nc.scalar.activation(e_sb, work, AF.Exp)
        sm = esmp.tile([1, NG], f32, tag="sm", name=f"sm{rep}_{c}")
        nc.vector.reduce_sum(out=sm, in_=e3, axis=mybir.AxisListType.X)
        rc = esmp.tile([1, NG], f32, tag="rc", name=f"rc{rep}_{c}")
        nc.vector.reciprocal(rc, sm)
        nc.vector.tensor_scalar_mul(rc, rc, PSCALE / H)
        attn_sb = work
        nc.vector.tensor_tensor(
            out=w3, in0=e3, in1=rc[:, :, None].to_broadcast((1, NG, S)),
            op=OP.mult)
        # broadcast attn across partitions via DRAM round-trip
        attn_dram = dpool.tile([1, NB], f32, tag="attn_dram")
        nc.gpsimd.dma_start(out=attn_dram[:, :], in_=attn_sb)
        attn_bc = bass.AP(
            tensor=attn_dram.tensor,
            offset=attn_dram.offset,
            ap=[[0, 128]] + [list(p) for p in attn_dram[:, :].ap[1:]],
        )
        ab_sb = abp.tile([128, NB], f32, tag="ab", name=f"ab{rep}_{c}")
        nc.gpsimd.dma_start(out=ab_sb, in_=attn_bc)

        # -- partial = attn*PSCALE/H * Wh, in place (fp8), then bulk DMA --
        for fo in range(NFB):
            nc.vector.tensor_tensor(
                out=wh8[:, fo, :], in0=wh8[:, fo, :], in1=ab_sb, op=OP.mult)
        partial_c = dpool.tile([F, NB], f8, tag="partial")
        nc.gpsimd.dma_start(
            out=partial_c.rearrange("(o p) n -> p o n", p=128), in_=wh8)

        # -- reduce over heads: ReduceScatter along f-axis (fp8) --
        rs_c = dpool.tile([FSLICE, NB], f8, tag="rs", name=f"rs{rep}_{c}")
        nc.gpsimd.collective_compute(
            "ReduceScatter", OP.add,
            replica_groups=[list(range(H))],
            ins=[partial_c[:, :]], outs=[rs_c[:, :]])
        rs_list.append(rs_c)

    # finish phase after all chunks (overlaps trailing collectives)
    for c in range(NCHUNK):
        rs_c = rs_list[c]
        rsb = rsbp.tile([128, FSLICE // 128, NB], f8, tag="rsb",
                        name=f"rsb{rep}_{c}")
        nc.sync.dma_start(
            out=rsb, in_=rs_c.rearrange("(o p) n -> p o n", p=128))
        rsc = rsbp.tile([128, FSLICE // 128, NB], bf16, tag="rsc",
                        name=f"rsc{rep}_{c}")
        for j in range(FSLICE // 128):
            nc.scalar.activation(
                rsc[:, j, :], rsb[:, j, :], AF.Identity,
                scale=1.0 / PSCALE, bias=bm_sb[:, j:j + 1])
        xrs = xrsp.tile([128, NB // 128, FSLICE], f32, tag="xrs")
        nc.sync.dma_start(
            out=xrs,
            in_=xres_in[c * NB:(c + 1) * NB, :].rearrange(
                "(o p) f -> p o f", p=128))
        obl = outstp.tile([128, NB // 128, FSLICE], f32, tag="obl")
        for bsub in range(NB // 128):
            for j in range(FSLICE // 128):
                pt = tpool.tile([128, 128], bf16, tag="tp")
                nc.tensor.transpose(
                    pt, rsc[:, j, bsub * 128:(bsub + 1) * 128], ident_bf)
                nc.vector.tensor_tensor(
                    out=obl[:, bsub, j * 128:(j + 1) * 128], in0=pt,
                    in1=xrs[:, bsub, j * 128:(j + 1) * 128], op=OP.add)
        nc.gpsimd.dma_start(
            out=out_ext[c * NB:(c + 1) * NB, :].rearrange(
                "(o p) f -> p o f", p=128),
            in_=obl)


def _get_nc():
    if "nc" not in _cache:
        _cache["nc"] = _build()
    return _cache["nc"]


def _make_in_maps(inputs_dict):
    import ml_dtypes
    f8 = ml_dtypes.float8_e4m3
    bf = ml_dtypes.bfloat16

    x = np.ascontiguousarray(
        np.asarray(inputs_dict["inputs"], dtype=np.float32).reshape(BS, F))
    W = np.asarray(inputs_dict["W"], dtype=np.float32)
    att_w = np.asarray(inputs_dict["att_w"], dtype=np.float32)
    bias = np.asarray(inputs_dict["bias"], dtype=np.float32)

    bm_full = bias.mean(axis=0)  # [F]

    # selector: 2.0 at (parity*64 + bs//S) % 128, duplicated along the
    # DoubleRow pair axis (X0q is likewise duplicated: 16 * 2 * 2 = 64)
    sel = np.zeros((2, 2, 128, NB), np.float32)
    for par in range(2):
        for j in range(NB):
            sel[par, :, par * 64 + j // S, j] = 2.0
    sel = sel.astype(f8)

    # X transposed: [F, BS] -> [128 fi_in, NKP k, 2 two, BS]
    # fi = (2k + two)*128 + fi_in
    xT8 = x.T.astype(f8)
    xt = np.ascontiguousarray(
        xT8.reshape(NKP, 2, 128, BS).transpose(2, 0, 1, 3))
    x0T8 = x[0::S, :].T.astype(f8)   # [F, B]
    x0t = np.ascontiguousarray(
        x0T8.reshape(NKP, 2, 128, B).transpose(2, 0, 1, 3))

    W8 = (W * np.float32(WSCALE)).astype(f8)  # [H, 2F, F]

    in_maps = []
    for i in range(H):
        W1 = W8[i, :F]   # [fi, fo]
        W2 = W8[i, F:]
        # w2t: [128 fi_in, NKP, 2, NFB fo_out, 128 fo_in]
        w2t = np.ascontiguousarray(
            W2.reshape(NKP, 2, 128, NFB, 128).transpose(2, 0, 1, 3, 4))
        # w1t: [4 fo4, 128 fi_in, NKP, 2, 512 fo_in]
        w1t = np.ascontiguousarray(
            W1.reshape(NKP, 2, 128, 4, 512).transpose(3, 2, 0, 1, 4))
        aw = att_w[i]  # [F]; fo = fo_out*128 + fo_in
        attw8 = np.zeros((128, NFB, 16), f8)
        attw8[:, :, 0] = (aw * np.float32(WSCALE)).astype(f8).reshape(NFB, 128).T
        attwb = np.ascontiguousarray(aw.astype(bf).reshape(NFB, 128).T)
        in_maps.append({
            "w2t": w2t,
            "w1t": w1t,
            "xt": xt,
            "x0t": x0t,
            "attw8": attw8,
            "attwb": attwb,
            "sel4": sel,
            "xres": np.ascontiguousarray(x[:, FSLICE * i:FSLICE * (i + 1)]),
            "bm": np.ascontiguousarray(bm_full[FSLICE * i:FSLICE * (i + 1)]),
        })
    return in_maps


def kernel(inputs, W, att_w, bias):
    from concourse.bass_utils import run_bass_kernel_spmd

    nc = _get_nc()
    in_maps = _make_in_maps(
        {"inputs": inputs, "W": W, "att_w": att_w, "bias": bias})
    res = run_bass_kernel_spmd(nc, in_maps, list(range(H)))
    _cache["last_result"] = res

    out = np.concatenate([res.results[i]["out"] for i in range(H)], axis=1)
    return out.reshape(B, S, F)
